# revision 30
# baseline (speedup 1.0000x reference)
"""Point-transformer block kernel for TRN2 (8-core data-parallel).

Core i handles serialized patches 2i,2i+1: rows = order[2048i:2048(i+1)].

CPE uses the ~17% sparsity of the 3x3x3 neighbor taps: the host ships,
per core, the *valid* (point, offset) pairs only — pre-gathered neighbor
features in feature-major layout, k-grouped and padded to 128-pair
chunks. Stage 1 projects each chunk with its offset's weights (PE);
stage 2 accumulates the projected rows into h via dma_scatter_add
(race-free: within one k-group every destination row is distinct).
The center tap (k=13, always valid, identity-aligned) is computed
densely into SBUF. h = center + gather_transpose(scattered part).

Activations feature-major (FM): X^T [128 (c%128), CC (c//128), rows];
matmuls lhsT=W^T-arranged weights. bf16 matmuls, f32 residual.
"""
from contextlib import ExitStack

import numpy as np
import ml_dtypes

import concourse.bacc as bacc
import concourse.bass as bass
import concourse.mybir as mybir
import concourse.tile as tile

P = 128
C = 512
CC = C // P
NH = 8
HD = 64
KP = 1024
R = 2048
NPATCH = R // KP
NKK = 27
KCENTER = 13
NFULL = 16384
EPS = 1e-5
SCALE = (C // NH) ** -0.5
F32 = mybir.dt.float32
BF16 = mybir.dt.bfloat16
F8 = mybir.dt.float8e4
I16 = mybir.dt.int16
AF = mybir.ActivationFunctionType
OP = mybir.AluOpType
DR = mybir.MatmulPerfMode.DoubleRow
WS = 16.0      # fp8 weight pre-scale (host multiplies weights by WS)
VD = 72        # v head width padded for DoubleRow (HD + denominator + pad)

HALF = 1024
NHALF = R // HALF
N512 = HALF // 512

HROWS = 17 * P          # h_dram rows: 2048 real + 128 trash/padding
TRASH = R               # scatter destination for padded pairs


def compute_nchunk(neighbor_idx, order):
    """Unified per-offset chunk counts (max over cores, 128-pair chunks)."""
    nbr = np.asarray(neighbor_idx)
    order = np.asarray(order)
    nchunk = {}
    for k in range(NKK):
        if k == KCENTER:
            continue
        mx = 0
        for c in range(8):
            rows = order[c * R:(c + 1) * R]
            mx = max(mx, int((nbr[rows, k] >= 0).sum()))
        if mx > 0:
            nchunk[k] = (mx + P - 1) // P
    return nchunk


def input_dram_specs(nchunk):
    """(name, shape, dtype) for every ExternalInput tensor."""
    ncpad = sum(nchunk.values()) * P
    specs = [
        ("featT_own", [P, CC, R], F32),
        ("u_nc", [P, CC, ncpad], BF16),
        ("sca_idx", [P, ncpad // 16], I16),
        ("ident_idx", [P, R // 16], I16),
        ("wcat", [NKK, C, C], BF16),
        ("lin_wT", [C, C], BF16),
        ("qkv_wT", [C, 3 * C], F8),
        ("proj_wT", [C, C], F8),
        ("fc1_wT", [C, 4 * C], F8),
        ("fc2_wT", [4 * C, C], F8),
    ]
    for nm, n in [("cpe_b", CC), ("lin_b", CC), ("cpe_ln_g", CC), ("cpe_ln_b", CC),
                  ("ln1_g", CC), ("ln1_b", CC), ("ln2_g", CC), ("ln2_b", CC),
                  ("q_b", CC), ("k_b", CC), ("proj_b", CC),
                  ("fc1_b", 4 * CC), ("fc2_b", CC)]:
        specs.append((nm, [P, n], F32))
    specs.append(("v_b_rep", [P, C], F32))
    return specs


def build_program(nchunk, gelu_exact=True, debug_taps=False):
    nc = bacc.Bacc("TRN2", target_bir_lowering=False, debug=False)

    dbg = {}

    def tap(name, ap):
        if not debug_taps:
            return
        t = nc.dram_tensor(f"dbg_{name}", list(ap.shape), ap.dtype,
                           kind="ExternalOutput")
        nc.sync.dma_start(t[:], ap)
        dbg[name] = t

    dram = {}
    for nm, shp, dt in input_dram_specs(nchunk):
        dram[nm] = nc.dram_tensor(nm, shp, dt, kind="ExternalInput")
    featT_own = dram["featT_own"]
    u_nc = dram["u_nc"]
    sca_idx = dram["sca_idx"]
    ident_idx = dram["ident_idx"]
    wcat = dram["wcat"]
    lin_wT, qkv_wT, proj_wT = dram["lin_wT"], dram["qkv_wT"], dram["proj_wT"]
    fc1_wT, fc2_wT = dram["fc1_wT"], dram["fc2_wT"]
    v_b_rep = dram["v_b_rep"]
    pvec_names = ["cpe_b", "lin_b", "cpe_ln_g", "cpe_ln_b", "ln1_g", "ln1_b",
                  "ln2_g", "ln2_b", "q_b", "k_b", "proj_b", "fc1_b", "fc2_b"]

    h_dram = nc.dram_tensor("h_scratch", [HROWS, C], BF16, kind="Internal")
    outT = nc.dram_tensor("outT", [P, CC, R], F32, kind="ExternalOutput")

    with tile.TileContext(nc) as tc, ExitStack() as ctx:
        pers = ctx.enter_context(tc.tile_pool(name="pers", bufs=1))
        resid = ctx.enter_context(tc.tile_pool(name="resid", bufs=2))

        pv = {}
        for nm in pvec_names:
            t = pers.tile(list(dram[nm].shape), F32, tag=f"pv_{nm}")
            nc.sync.dma_start(t[:], dram[nm][:])
            pv[nm] = t
        v_b_t = pers.tile([P, C], F32, tag="v_b")
        nc.sync.dma_start(v_b_t[:], v_b_rep[:])
        ones_bf = pers.tile([P, P], BF16, tag="ones_bf")
        nc.vector.memset(ones_bf[:], 1.0)
        ones_f = pers.tile([P, P], F32, tag="ones_f")
        nc.vector.memset(ones_f[:], 1.0)
        ones1_bf = pers.tile([1, HD], BF16, tag="ones1_bf")
        nc.vector.memset(ones1_bf[:], 1.0)
        eps_t = pers.tile([P, 1], F32, tag="eps_t")
        nc.vector.memset(eps_t[:], EPS)

        def wload(pool, dram_ap, kdim, ndim, tag, dt=BF16):
            t = pool.tile([P, kdim // P, ndim], dt, tag=tag)
            nc.sync.dma_start(t[:], dram_ap.rearrange("(ko ki) n -> ki ko n", ki=P))
            return t

        def fm_ln_stats(lnp, x, x_is_f32):
            with tc.tile_pool(name="ln_ps", bufs=1, space="PSUM") as lps:
                sums_ps = lps.tile([P, R], F32, tag="ln_sums")
                sqs_ps = lps.tile([P, R], F32, tag="ln_sqs")
                for half in range(NHALF):
                    o = half * HALF
                    sq = lnp.tile([P, CC, HALF], BF16, tag="ln_sq")
                    nc.scalar.activation(sq[:], x[:, :, o:o + HALF], AF.Square)
                    if x_is_f32:
                        # bf16 shadow so the sums matmul runs at 1 cyc/row
                        xb = lnp.tile([P, CC, HALF], BF16, tag="ln_xb")
                        nc.vector.tensor_copy(xb[:], x[:, :, o:o + HALF])
                    for kc in range(CC):
                        for nn in range(N512):
                            sl = slice(o + nn * 512, o + (nn + 1) * 512)
                            sli = slice(nn * 512, (nn + 1) * 512)
                            xsrc = xb[:, kc, sli] if x_is_f32 else x[:, kc, sl]
                            nc.tensor.matmul(sums_ps[:, sl], ones_bf[:], xsrc,
                                             start=(kc == 0), stop=(kc == CC - 1))
                            nc.tensor.matmul(sqs_ps[:, sl], ones_bf[:],
                                             sq[:, kc, sli],
                                             start=(kc == 0), stop=(kc == CC - 1))
                neg_m = lnp.tile([P, R], F32, tag="ln_negm")
                nc.vector.tensor_scalar(neg_m[:], sums_ps[:], -1.0 / C, None,
                                        op0=OP.mult)
                msq = lnp.tile([P, R], F32, tag="ln_tmp")
                nc.scalar.activation(msq[:], neg_m[:], AF.Square)
                var = lnp.tile([P, R], F32, tag="ln_tmp2")
                nc.vector.scalar_tensor_tensor(var[:], sqs_ps[:], 1.0 / C, msq[:],
                                               op0=OP.mult, op1=OP.subtract)
            std = lnp.tile([P, R], F32, tag="ln_tmp")
            nc.scalar.activation(std[:], var[:], AF.Sqrt, bias=eps_t[:])
            inv_std = lnp.tile([P, R], F32, tag="ln_istd")
            nc.vector.reciprocal_approx_fast(inv_std[:], std[:])
            return neg_m, inv_std

        def fm_ln_apply(lnp, x, neg_m, inv_std, g, b, out, m, res=None):
            t = lnp.tile([P, R], F32, tag="ln_t")
            nc.vector.tensor_tensor(t[:], x[:, m, :], neg_m[:], op=OP.add)
            nc.vector.tensor_tensor(t[:], t[:], inv_std[:], op=OP.mult)
            if res is None:
                nc.vector.tensor_scalar(out[:, m, :], t[:], g[:, m:m + 1],
                                        b[:, m:m + 1], op0=OP.mult, op1=OP.add)
            else:
                nc.vector.tensor_scalar(t[:], t[:], g[:, m:m + 1], b[:, m:m + 1],
                                        op0=OP.mult, op1=OP.add)
                nc.vector.tensor_tensor(out[:, m, :], t[:], res[:, m, :], op=OP.add)

        feat1 = None

        # =========================== CPE =================================
        with tc.tile_pool(name="hpool", bufs=1) as hpool:
            # zero the scatter accumulator
            with tc.tile_pool(name="zinit", bufs=1) as zp:
                zt = zp.tile([P, HROWS // P, C], BF16, tag="zt")
                nc.vector.memset(zt[:], 0.0)
                nc.sync.dma_start(
                    h_dram[:].rearrange("(a ki) e -> ki a e", ki=P), zt[:])

            fown = hpool.tile([P, CC, R], F32, tag="fown")
            nc.sync.dma_start(fown[:], featT_own[:])
            h1_sb = hpool.tile([P, CC, R], BF16, tag="h1")

            with tc.tile_pool(name="hbp", bufs=1) as hbp:
                h_base = hbp.tile([P, CC, R], BF16, tag="h_base")
                # ---- center tap: dense, identity-aligned, stays in SBUF ----
                with tc.tile_pool(name="w13p", bufs=1) as w13p, \
                     tc.tile_pool(name="cps", bufs=1, space="PSUM") as cps:
                    featbf = w13p.tile([P, CC, R], BF16, tag="featbf")
                    nc.vector.tensor_copy(featbf[:], fown[:])
                    w13 = wload(w13p, wcat[KCENTER], C, C, "w13")
                    for m in range(CC):
                        for g in range(CC):
                            ps = cps.tile([P, 512], F32, tag=f"cps{(m * CC + g) % 4}")
                            for kc in range(CC):
                                nc.tensor.matmul(
                                    ps[:], w13[:, kc, m * P:(m + 1) * P],
                                    featbf[:, kc, g * 512:(g + 1) * 512],
                                    start=(kc == 0), stop=(kc == CC - 1))
                            nc.vector.tensor_copy(
                                h_base[:, m, g * 512:(g + 1) * 512], ps[:])

                # ---- sparse taps: project k-group chunks, scatter-add ----
                ncpad = sum(nchunk.values()) * P
                si = hbp.tile([P, ncpad // 16], I16, tag="si")
                nc.sync.dma_start(si[:], sca_idx[:])
                # split the k-groups into ~4 u-load pieces (SBUF pressure)
                items = list(nchunk.items())
                total = sum(nk for _, nk in items)
                target = (total + 5) // 6
                groups, cur, acc = [], [], 0
                for k, nk in items:
                    cur.append((k, nk))
                    acc += nk
                    if acc >= target:
                        groups.append(cur)
                        cur, acc = [], 0
                if cur:
                    groups.append(cur)
                with (
                    tc.tile_pool(name="upool", bufs=2) as up,
                    tc.tile_pool(name="wstream", bufs=3) as wp,
                    tc.tile_pool(name="zbp", bufs=3) as zbp,
                    tc.tile_pool(name="cps2", bufs=2, space="PSUM") as cps2,
                ):
                    off = 0
                    for grp in groups:
                        gsize = sum(nk for _, nk in grp)
                        u_t = up.tile([P, CC, gsize * P], BF16, tag="u_t")
                        nc.sync.dma_start(
                            u_t[:], u_nc[:, :, off:off + gsize * P])
                        loff = 0
                        for k, nk in grp:
                            w_t = wload(wp, wcat[k], C, C, "w_t")
                            zb = zbp.tile([P, nk, C], BF16, tag="zb")
                            for j in range(nk):
                                ps = cps2.tile([P, C], F32, tag=f"ncps{j % 4}")
                                for kc in range(CC):
                                    nc.tensor.matmul(
                                        ps[:], u_t[:, kc, loff + j * P:
                                                   loff + (j + 1) * P],
                                        w_t[:, kc, :],
                                        start=(kc == 0), stop=(kc == CC - 1))
                                nc.vector.tensor_copy(zb[:, j, :], ps[:])
                            nc.gpsimd.dma_scatter_add(
                                h_dram[:], zb[:],
                                si[:, off // 16:(off + nk * P) // 16],
                                nk * P, nk * P, C, single_packet=False)
                            off += nk * P
                            loff += nk * P

                # ---- readback (transposed to FM) + combine + bias ----
                with tc.tile_pool(name="hrp", bufs=1) as hrp:
                    idt = hrp.tile([P, R // 16], I16, tag="idt")
                    nc.sync.dma_start(idt[:], ident_idx[:])
                    hr = hrp.tile([P, CC, R], BF16, tag="h_rest")
                    nc.gpsimd.dma_gather(hr[:], h_dram[:], idt[:], R, R, C,
                                         transpose=True, single_packet=False)
                    for m in range(CC):
                        nc.vector.scalar_tensor_tensor(
                            h1_sb[:, m, :], h_base[:, m, :],
                            pv["cpe_b"][:, m:m + 1],
                            hr[:, m, :], op0=OP.add, op1=OP.add)
                    tap("h1", h1_sb[:])

            # ---- cpe linear ----
            h2_sb = hpool.tile([P, CC, R], BF16, tag="h2")
            with (
                tc.tile_pool(name="linw", bufs=1) as lwp,
                tc.tile_pool(name="lin_ps", bufs=1, space="PSUM") as lps,
            ):
                lin_w_t = wload(lwp, lin_wT[:], C, C, "lin_w")
                for half in range(NHALF):
                    hp = [lps.tile([P, HALF], F32, tag=f"mm_ps{m}", name=f"lin_ps{m}") for m in range(CC)]
                    for kc in range(CC):
                        for m in range(CC):
                            for nn in range(N512):
                                sl = slice(nn * 512, (nn + 1) * 512)
                                hsl = slice(half * HALF + nn * 512,
                                            half * HALF + (nn + 1) * 512)
                                nc.tensor.matmul(
                                    hp[m][:, sl],
                                    lin_w_t[:, kc, m * P:(m + 1) * P],
                                    h1_sb[:, kc, hsl],
                                    start=(kc == 0), stop=(kc == CC - 1))
                    for m in range(CC):
                        nc.vector.tensor_scalar(
                            h2_sb[:, m, half * HALF:(half + 1) * HALF], hp[m][:],
                            pv["lin_b"][:, m:m + 1], None, op0=OP.add)

            feat1 = resid.tile([P, CC, R], F32, tag="resid")
            with tc.tile_pool(name="lnp1", bufs=1) as lnp:
                neg_m, inv_std = fm_ln_stats(lnp, h2_sb, x_is_f32=False)
                for m in range(CC):
                    fm_ln_apply(lnp, h2_sb, neg_m, inv_std, pv["cpe_ln_g"],
                                pv["cpe_ln_b"], feat1, m, res=fown)
            tap("feat1", feat1[:])

        # ===================== ln1 + qkv + attn + proj ===================
        with tc.tile_pool(name="attn", bufs=1) as ap_:
            q_sb = ap_.tile([P, CC, R], BF16, tag="q_sb")
            k_sb = ap_.tile([P, CC, R], BF16, tag="k_sb")
            v_sb = ap_.tile([P, R // P, NH, VD], F8, tag="v_sb")
            o_sb = ap_.tile([P, CC, R], F8, tag="o_sb")   # holds WS*o
            nc.vector.memset(v_sb[:], 1.0 / WS)           # denom col -> rec=WS/d

            with tc.tile_pool(name="x1p", bufs=1) as x1p:
                with tc.tile_pool(name="lnp2", bufs=1) as lnp:
                    neg_m, inv_std = fm_ln_stats(lnp, feat1, x_is_f32=True)
                    x1 = x1p.tile([P, CC, R], F8, tag="x1")
                    for m in range(CC):
                        fm_ln_apply(lnp, feat1, neg_m, inv_std, pv["ln1_g"],
                                    pv["ln1_b"], x1, m)
                    tap("x1", x1[:])

                with tc.tile_pool(name="qkv_ps", bufs=1, space="PSUM") as qps, \
                     tc.tile_pool(name="qkvw", bufs=2) as qwp:
                    for half in range(NHALF):
                        o = half * HALF
                        # q_b is host-scaled by SCALE already
                        for part, dst, bias, scl in [
                                (0, q_sb, pv["q_b"], SCALE / WS),
                                (1, k_sb, pv["k_b"], 1.0 / WS)]:
                            qkv_w_t = wload(qwp, qkv_wT[:, part * C:(part + 1) * C],
                                            C, C, "qkv_w_part", dt=F8)
                            pp_ = [qps.tile([P, HALF], F32, tag=f"mm_ps{m}", name=f"qkv_ps{m}")
                                   for m in range(CC)]
                            for kc in range(0, CC, 2):
                                for m in range(CC):
                                    for nn in range(N512):
                                        sl = slice(nn * 512, (nn + 1) * 512)
                                        nc.tensor.matmul(
                                            pp_[m][:, sl],
                                            qkv_w_t[:, kc:kc + 2,
                                                    m * P:(m + 1) * P],
                                            x1[:, kc:kc + 2, o + nn * 512:
                                               o + (nn + 1) * 512],
                                            start=(kc == 0), stop=(kc == CC - 2),
                                            perf_mode=DR)
                            for m in range(CC):
                                nc.vector.tensor_scalar(
                                    dst[:, m, o:o + HALF], pp_[m][:],
                                    bias[:, m:m + 1], scl,
                                    op0=OP.add, op1=OP.mult)
                        v_w_t = wload(qwp, qkv_wT[:, 2 * C:3 * C], C, C,
                                      "qkv_w_part", dt=F8)
                        for rc in range(HALF // P):
                            row0 = o + rc * P
                            vp = qps.tile([P, C], F32, tag=f"mm_ps{rc % 2}",
                                          name="vp")
                            for kc in range(0, CC, 2):
                                nc.tensor.matmul(
                                    vp[:], x1[:, kc:kc + 2, row0:row0 + P],
                                    v_w_t[:, kc:kc + 2, :],
                                    start=(kc == 0), stop=(kc == CC - 2),
                                    perf_mode=DR)
                            nc.vector.scalar_tensor_tensor(
                                v_sb[:, row0 // P, :, :HD],
                                vp[:].rearrange("p (h d) -> p h d", d=HD),
                                1.0 / WS,
                                v_b_t[:].rearrange("p (h d) -> p h d", d=HD),
                                op0=OP.mult, op1=OP.add)
                    tap("q", q_sb[:])
                    tap("k", k_sb[:])
                    tap("v", v_sb[:])

            with (
                tc.tile_pool(name="pT_pool", bufs=2) as ptp,
                tc.tile_pool(name="at_ps", bufs=1, space="PSUM") as aps,
                tc.tile_pool(name="at_ps2", bufs=2, space="PSUM") as aps2,
            ):
                for pt in range(NPATCH):
                    po = pt * KP
                    for h in range(NH):
                        hc, hpo = divmod(h * HD, P)
                        pT = ptp.tile([P, KP // P, KP], F8, tag="pT")
                        for jc in range(KP // P):
                            sps = aps2.tile([P, KP], F32, tag="s_ps")
                            for nn in range(KP // 512):
                                nc.tensor.matmul(
                                    sps[:, nn * 512:(nn + 1) * 512],
                                    k_sb[hpo:hpo + HD, hc,
                                         po + jc * P:po + (jc + 1) * P],
                                    q_sb[hpo:hpo + HD, hc,
                                         po + nn * 512:po + (nn + 1) * 512],
                                    start=True, stop=True)
                            nc.scalar.activation(pT[:, jc, :], sps[:], AF.Exp)
                        ops_ = aps.tile([P, KP], F32, tag="o_ps")
                        for jc in range(0, KP // P, 2):
                            for nn in range(KP // 512):
                                sl = slice(nn * 512, (nn + 1) * 512)
                                nc.tensor.matmul(
                                    ops_[:VD, sl],
                                    v_sb[:, (po + jc * P) // P:
                                         (po + jc * P) // P + 2, h, :],
                                    pT[:, jc:jc + 2, sl],
                                    start=(jc == 0), stop=(jc == KP // P - 2),
                                    perf_mode=DR)
                        # free the O psum bank early: copy to SBUF, then
                        # run the normalize tail off the PE critical path
                        ocp = ptp.tile([HD, KP], BF16, tag="ocp")
                        nc.vector.tensor_copy(ocp[:], ops_[:HD, :])
                        dcp = ptp.tile([1, KP], F32, tag="dcp")
                        nc.vector.tensor_copy(dcp[:], ops_[HD:HD + 1, :])
                        rec = ptp.tile([1, KP], F32, tag="rec")
                        nc.vector.reciprocal_approx_fast(rec[:], dcp[:])
                        rec_bf = ptp.tile([1, KP], BF16, tag="rec_bf")
                        nc.vector.tensor_copy(rec_bf[:], rec[:])
                        rps = aps.tile([HD, KP], F32, tag="rec_ps")
                        for nn in range(KP // 512):
                            sl = slice(nn * 512, (nn + 1) * 512)
                            nc.tensor.matmul(rps[:, sl], ones1_bf[:],
                                             rec_bf[:, sl], start=True, stop=True)
                        nc.vector.tensor_tensor(
                            o_sb[hpo:hpo + HD, hc, po:po + KP],
                            ocp[:], rps[:], op=OP.mult)
                tap("o", o_sb[:])

            feat2 = resid.tile([P, CC, R], F32, tag="resid")
            with (
                tc.tile_pool(name="projw", bufs=1) as pwp,
                tc.tile_pool(name="proj_tp", bufs=2) as ptp2,
                tc.tile_pool(name="proj_ps", bufs=1, space="PSUM") as pps,
            ):
                proj_w_t = wload(pwp, proj_wT[:], C, C, "proj_w", dt=F8)
                for half in range(NHALF):
                    o = half * HALF
                    pp_ = [pps.tile([P, HALF], F32, tag=f"mm_ps{m}", name=f"proj_ps{m}")
                           for m in range(CC)]
                    for kc in range(0, CC, 2):
                        for m in range(CC):
                            for nn in range(N512):
                                sl = slice(nn * 512, (nn + 1) * 512)
                                nc.tensor.matmul(
                                    pp_[m][:, sl],
                                    proj_w_t[:, kc:kc + 2, m * P:(m + 1) * P],
                                    o_sb[:, kc:kc + 2,
                                         o + nn * 512:o + (nn + 1) * 512],
                                    start=(kc == 0), stop=(kc == CC - 2),
                                    perf_mode=DR)
                    for m in range(CC):
                        tp_ = ptp2.tile([P, HALF], BF16, tag=f"tp{m % 2}")
                        nc.scalar.activation(
                            tp_[:], pp_[m][:], AF.Identity,
                            bias=pv["proj_b"][:, m:m + 1], scale=1.0 / (WS * WS))
                        nc.vector.tensor_tensor(
                            feat2[:, m, o:o + HALF], tp_[:],
                            feat1[:, m, o:o + HALF], op=OP.add)
            tap("feat2", feat2[:])

        # =============================== MLP =============================
        QH = 512  # row quarter
        with tc.tile_pool(name="mlp", bufs=1) as mp_:
            x2 = mp_.tile([P, CC, R], F8, tag="x2")
            with tc.tile_pool(name="lnp3", bufs=1) as lnp:
                neg_m, inv_std = fm_ln_stats(lnp, feat2, x_is_f32=True)
                for m in range(CC):
                    fm_ln_apply(lnp, feat2, neg_m, inv_std, pv["ln2_g"],
                                pv["ln2_b"], x2, m)

            gelu_f = AF.Gelu if gelu_exact else AF.Tanh
            fc1_w_t = wload(mp_, fc1_wT[:], C, 4 * C, "fc1_w", dt=F8)
            fc2_w_t = wload(mp_, fc2_wT[:], 4 * C, C, "fc2_w", dt=F8)
            with (
                tc.tile_pool(name="g_pool", bufs=2) as gp_,
                tc.tile_pool(name="out_pool", bufs=2) as op_,
                tc.tile_pool(name="mlp_tp", bufs=2) as mtp,
                tc.tile_pool(name="mlp_ps", bufs=2, space="PSUM") as mps,
            ):
                for quarter in range(R // QH):
                    o = quarter * QH
                    g_sb = gp_.tile([P, 4 * CC, QH], F8, tag="g_sb")
                    for mg in range(4):
                        fp = [mps.tile([P, QH], F32, tag=f"mm_ps{m}", name=f"mlp_ps{m}")
                              for m in range(CC)]
                        for kc in range(0, CC, 2):
                            for m in range(CC):
                                mm = mg * CC + m
                                nc.tensor.matmul(
                                    fp[m][:],
                                    fc1_w_t[:, kc:kc + 2, mm * P:(mm + 1) * P],
                                    x2[:, kc:kc + 2, o:o + QH],
                                    start=(kc == 0), stop=(kc == CC - 2),
                                    perf_mode=DR)
                        for m in range(CC):
                            mm = mg * CC + m
                            nc.scalar.activation(g_sb[:, mm, :], fp[m][:], gelu_f,
                                                 bias=pv["fc1_b"][:, mm:mm + 1],
                                                 scale=1.0 / WS)
                    f2 = [mps.tile([P, QH], F32, tag=f"mm_ps{m}", name=f"mlp_ps{m}") for m in range(CC)]
                    for kc in range(0, 4 * CC, 2):
                        for m in range(CC):
                            nc.tensor.matmul(
                                f2[m][:],
                                fc2_w_t[:, kc:kc + 2, m * P:(m + 1) * P],
                                g_sb[:, kc:kc + 2, :],
                                start=(kc == 0), stop=(kc == 4 * CC - 2),
                                perf_mode=DR)
                    out_q = op_.tile([P, CC, QH], F32, tag="out_q")
                    for m in range(CC):
                        tq_ = mtp.tile([P, QH], BF16, tag=f"tq{m % 2}")
                        nc.scalar.activation(
                            tq_[:], f2[m][:], AF.Identity,
                            bias=pv["fc2_b"][:, m:m + 1], scale=1.0 / WS)
                        nc.vector.tensor_tensor(
                            out_q[:, m, :], tq_[:],
                            feat2[:, m, o:o + QH], op=OP.add)
                    nc.sync.dma_start(outT[:, :, o:o + QH], out_q[:])

    nc.compile()
    return nc


# ====================== host-side preparation ======================

def prep_shared(inputs):
    f32 = np.float32
    bf = ml_dtypes.bfloat16
    f8 = mybir.dt.np(F8)
    ws = np.float32(WS)

    def pp(v):
        return np.ascontiguousarray(np.asarray(v, f32).reshape(-1, P).T)

    def w8(v):  # fp8 weight, pre-scaled so values sit in e4m3's normal range
        return np.ascontiguousarray(np.asarray(v, f32).T * ws).astype(f8)

    qkv_b = np.asarray(inputs["qkv_b"], f32)
    ident = np.arange(R, dtype=np.int16).reshape(-1, 16).T  # [16, R//16]
    return dict(
        ident_idx=np.ascontiguousarray(np.tile(ident, (P // 16, 1))),
        wcat=np.ascontiguousarray(
            np.transpose(np.asarray(inputs["cpe_w"], f32), (0, 2, 1))).astype(bf),
        lin_wT=np.ascontiguousarray(np.asarray(inputs["cpe_lin_w"], f32).T).astype(bf),
        qkv_wT=w8(inputs["qkv_w"]),
        proj_wT=w8(inputs["proj_w"]),
        fc1_wT=w8(inputs["fc1_w"]),
        fc2_wT=w8(inputs["fc2_w"]),
        cpe_b=pp(inputs["cpe_b"]), lin_b=pp(inputs["cpe_lin_b"]),
        cpe_ln_g=pp(inputs["cpe_ln_g"]), cpe_ln_b=pp(inputs["cpe_ln_b"]),
        ln1_g=pp(inputs["ln1_g"]), ln1_b=pp(inputs["ln1_b"]),
        ln2_g=pp(inputs["ln2_g"]), ln2_b=pp(inputs["ln2_b"]),
        q_b=pp(qkv_b[:C] * WS), k_b=pp(qkv_b[C:2 * C] * WS),
        v_b_rep=np.ascontiguousarray(np.broadcast_to(qkv_b[2 * C:], (P, C))),
        proj_b=pp(inputs["proj_b"]),
        fc1_b=pp(inputs["fc1_b"]), fc2_b=pp(inputs["fc2_b"]),
    )


def prep_core(inputs, core, nchunk):
    f32 = np.float32
    bf = ml_dtypes.bfloat16
    order = np.asarray(inputs["order"])
    feat = np.asarray(inputs["feat"], f32)
    nbr = np.asarray(inputs["neighbor_idx"])
    rows = order[core * R:(core + 1) * R]

    featT_own = np.ascontiguousarray(
        feat[rows].T.reshape(CC, P, R).transpose(1, 0, 2))

    nb = nbr[rows]
    srcs, dsts = [], []
    for k, nk in nchunk.items():
        v = np.nonzero(nb[:, k] >= 0)[0]
        src = np.full(nk * P, NFULL, np.int64)
        dst = np.full(nk * P, TRASH, np.int64)
        src[:len(v)] = nb[v, k]
        dst[:len(v)] = v
        srcs.append(src)
        dsts.append(dst)
    src_all = np.concatenate(srcs)
    dst_all = np.concatenate(dsts)

    featp = np.vstack([feat, np.zeros((1, C), f32)])
    u = featp[src_all]                                    # [NCPAD, C]
    u_fm = np.ascontiguousarray(
        u.T.reshape(CC, P, -1).transpose(1, 0, 2)).astype(bf)

    sca = dst_all.astype(np.int16).reshape(-1, 16).T      # [16, NCPAD//16]
    sca_idx = np.ascontiguousarray(np.tile(sca, (P // 16, 1)))
    return dict(featT_own=featT_own, u_nc=u_fm, sca_idx=sca_idx), rows


def unshard_out(res_outT):
    return np.ascontiguousarray(
        np.asarray(res_outT).transpose(1, 0, 2).reshape(C, R).T)


# ======================= public entry point =======================

_CACHED = {}


def get_program(inputs):
    """Build (or fetch) the program for these inputs' sparsity pattern."""
    nchunk = compute_nchunk(inputs["neighbor_idx"], inputs["order"])
    key = tuple(sorted(nchunk.items()))
    if key not in _CACHED:
        _CACHED[key] = build_program(nchunk)
    return _CACHED[key], nchunk


def kernel(**inputs) -> np.ndarray:
    """Full-input, full-output entry. Shards across 8 NeuronCores by
    serialized patches (2 per core), runs the Bass kernel, scatters the
    per-core outputs back to original point order."""
    from concourse.bass_utils import run_bass_kernel_spmd

    inputs = {k: np.asarray(v) for k, v in inputs.items()}
    nc, nchunk = get_program(inputs)
    sh = prep_shared(inputs)
    in_maps, rows_l = [], []
    for c in range(8):
        ci, rows = prep_core(inputs, c, nchunk)
        in_maps.append({**sh, **ci})
        rows_l.append(rows)

    res = None
    last_err = None
    for attempt in range(3):
        try:
            res = run_bass_kernel_spmd(nc, in_maps, core_ids=list(range(8))).results
            break
        except Exception as e:   # transient NRT/axon hiccups: retry
            last_err = e
            import time as _t
            _t.sleep(2.0)
    if res is None:
        raise last_err

    out = np.zeros((NFULL, C), np.float32)
    for c in range(8):
        out[rows_l[c]] = unshard_out(res[c]["outT"])
    return out


# revision 31
# speedup vs baseline: 44.2894x; 44.2894x over previous
"""Point-transformer block kernel for TRN2 (8-core data-parallel).

Core i handles serialized patches 2i,2i+1: rows = order[2048i:2048(i+1)].

CPE uses the ~17% sparsity of the 3x3x3 neighbor taps: the host ships,
per core, the *valid* (point, offset) pairs only — pre-gathered neighbor
features in feature-major layout, k-grouped and padded to 128-pair
chunks. Stage 1 projects each chunk with its offset's weights (PE);
stage 2 accumulates the projected rows into h via dma_scatter_add
(race-free: within one k-group every destination row is distinct).
The center tap (k=13, always valid, identity-aligned) is computed
densely into SBUF. h = center + gather_transpose(scattered part).

Activations feature-major (FM): X^T [128 (c%128), CC (c//128), rows];
matmuls lhsT=W^T-arranged weights. bf16 matmuls, f32 residual.
"""
from contextlib import ExitStack

import numpy as np
import ml_dtypes

import concourse.bacc as bacc
import concourse.bass as bass
import concourse.mybir as mybir
import concourse.tile as tile

P = 128
C = 512
CC = C // P
NH = 8
HD = 64
KP = 1024
R = 2048
NPATCH = R // KP
NKK = 27
KCENTER = 13
NFULL = 16384
EPS = 1e-5
SCALE = (C // NH) ** -0.5
F32 = mybir.dt.float32
BF16 = mybir.dt.bfloat16
F8 = mybir.dt.float8e4
I16 = mybir.dt.int16
AF = mybir.ActivationFunctionType
OP = mybir.AluOpType
DR = mybir.MatmulPerfMode.DoubleRow
WS = 16.0      # fp8 weight pre-scale (host multiplies weights by WS)
VD = 72        # v head width padded for DoubleRow (HD + denominator + pad)

HALF = 1024
NHALF = R // HALF
N512 = HALF // 512

HROWS = 17 * P          # h_dram rows: 2048 real + 128 trash/padding
TRASH = R               # scatter destination for padded pairs


def compute_nchunk(neighbor_idx, order):
    """Unified per-offset chunk counts (max over cores, 128-pair chunks)."""
    nbr = np.asarray(neighbor_idx)
    order = np.asarray(order)
    nchunk = {}
    for k in range(NKK):
        if k == KCENTER:
            continue
        mx = 0
        for c in range(8):
            rows = order[c * R:(c + 1) * R]
            mx = max(mx, int((nbr[rows, k] >= 0).sum()))
        if mx > 0:
            nchunk[k] = (mx + P - 1) // P
    return nchunk


def input_dram_specs(nchunk):
    """(name, shape, dtype) for every ExternalInput tensor."""
    ncpad = sum(nchunk.values()) * P
    specs = [
        ("featT_own", [P, CC, R], F32),
        ("u_nc", [P, CC, ncpad], BF16),
        ("sca_idx", [P, ncpad // 16], I16),
        ("ident_idx", [P, R // 16], I16),
        ("wcat", [NKK, C, C], BF16),
        ("lin_wT", [C, C], BF16),
        ("qkv_wT", [C, 3 * C], F8),
        ("proj_wT", [C, C], F8),
        ("fc1_wT", [C, 4 * C], F8),
        ("fc2_wT", [4 * C, C], F8),
    ]
    for nm, n in [("cpe_b", CC), ("lin_b", CC), ("cpe_ln_g", CC), ("cpe_ln_b", CC),
                  ("ln1_g", CC), ("ln1_b", CC), ("ln2_g", CC), ("ln2_b", CC),
                  ("q_b", CC), ("k_b", CC), ("proj_b", CC),
                  ("fc1_b", 4 * CC), ("fc2_b", CC)]:
        specs.append((nm, [P, n], F32))
    specs.append(("v_b_rep", [P, C], F32))
    return specs


def build_program(nchunk, gelu_exact=True, debug_taps=False):
    nc = bacc.Bacc("TRN2", target_bir_lowering=False, debug=False)

    dbg = {}

    def tap(name, ap):
        if not debug_taps:
            return
        t = nc.dram_tensor(f"dbg_{name}", list(ap.shape), ap.dtype,
                           kind="ExternalOutput")
        nc.sync.dma_start(t[:], ap)
        dbg[name] = t

    dram = {}
    for nm, shp, dt in input_dram_specs(nchunk):
        dram[nm] = nc.dram_tensor(nm, shp, dt, kind="ExternalInput")
    featT_own = dram["featT_own"]
    u_nc = dram["u_nc"]
    sca_idx = dram["sca_idx"]
    ident_idx = dram["ident_idx"]
    wcat = dram["wcat"]
    lin_wT, qkv_wT, proj_wT = dram["lin_wT"], dram["qkv_wT"], dram["proj_wT"]
    fc1_wT, fc2_wT = dram["fc1_wT"], dram["fc2_wT"]
    v_b_rep = dram["v_b_rep"]
    pvec_names = ["cpe_b", "lin_b", "cpe_ln_g", "cpe_ln_b", "ln1_g", "ln1_b",
                  "ln2_g", "ln2_b", "q_b", "k_b", "proj_b", "fc1_b", "fc2_b"]

    h_dram = nc.dram_tensor("h_scratch", [HROWS, C], BF16, kind="Internal")
    outT = nc.dram_tensor("outT", [P, CC, R], F32, kind="ExternalOutput")

    with tile.TileContext(nc) as tc, ExitStack() as ctx:
        pers = ctx.enter_context(tc.tile_pool(name="pers", bufs=1))
        resid = ctx.enter_context(tc.tile_pool(name="resid", bufs=2))

        pv = {}
        for nm in pvec_names:
            t = pers.tile(list(dram[nm].shape), F32, tag=f"pv_{nm}")
            nc.sync.dma_start(t[:], dram[nm][:])
            pv[nm] = t
        v_b_t = pers.tile([P, C], F32, tag="v_b")
        nc.sync.dma_start(v_b_t[:], v_b_rep[:])
        ones_bf = pers.tile([P, P], BF16, tag="ones_bf")
        nc.vector.memset(ones_bf[:], 1.0)
        ones_f = pers.tile([P, P], F32, tag="ones_f")
        nc.vector.memset(ones_f[:], 1.0)
        ones1_bf = pers.tile([1, HD], BF16, tag="ones1_bf")
        nc.vector.memset(ones1_bf[:], 1.0)
        eps_t = pers.tile([P, 1], F32, tag="eps_t")
        nc.vector.memset(eps_t[:], EPS)

        def wload(pool, dram_ap, kdim, ndim, tag, dt=BF16):
            t = pool.tile([P, kdim // P, ndim], dt, tag=tag)
            nc.sync.dma_start(t[:], dram_ap.rearrange("(ko ki) n -> ki ko n", ki=P))
            return t

        def fm_ln_stats(lnp, x, x_is_f32):
            with tc.tile_pool(name="ln_ps", bufs=1, space="PSUM") as lps:
                sums_ps = lps.tile([P, R], F32, tag="ln_sums")
                sqs_ps = lps.tile([P, R], F32, tag="ln_sqs")
                for half in range(NHALF):
                    o = half * HALF
                    sq = lnp.tile([P, CC, HALF], BF16, tag="ln_sq")
                    nc.scalar.activation(sq[:], x[:, :, o:o + HALF], AF.Square)
                    if x_is_f32:
                        # bf16 shadow so the sums matmul runs at 1 cyc/row
                        xb = lnp.tile([P, CC, HALF], BF16, tag="ln_xb")
                        nc.vector.tensor_copy(xb[:], x[:, :, o:o + HALF])
                    for kc in range(CC):
                        for nn in range(N512):
                            sl = slice(o + nn * 512, o + (nn + 1) * 512)
                            sli = slice(nn * 512, (nn + 1) * 512)
                            xsrc = xb[:, kc, sli] if x_is_f32 else x[:, kc, sl]
                            nc.tensor.matmul(sums_ps[:, sl], ones_bf[:], xsrc,
                                             start=(kc == 0), stop=(kc == CC - 1))
                            nc.tensor.matmul(sqs_ps[:, sl], ones_bf[:],
                                             sq[:, kc, sli],
                                             start=(kc == 0), stop=(kc == CC - 1))
                neg_m = lnp.tile([P, R], F32, tag="ln_negm")
                nc.vector.tensor_scalar(neg_m[:], sums_ps[:], -1.0 / C, None,
                                        op0=OP.mult)
                msq = lnp.tile([P, R], F32, tag="ln_tmp")
                nc.scalar.activation(msq[:], neg_m[:], AF.Square)
                var = lnp.tile([P, R], F32, tag="ln_tmp2")
                nc.vector.scalar_tensor_tensor(var[:], sqs_ps[:], 1.0 / C, msq[:],
                                               op0=OP.mult, op1=OP.subtract)
            std = lnp.tile([P, R], F32, tag="ln_tmp")
            nc.scalar.activation(std[:], var[:], AF.Sqrt, bias=eps_t[:])
            inv_std = lnp.tile([P, R], F32, tag="ln_istd")
            nc.vector.reciprocal_approx_fast(inv_std[:], std[:])
            return neg_m, inv_std

        def fm_ln_apply(lnp, x, neg_m, inv_std, g, b, out, m, res=None):
            t = lnp.tile([P, R], F32, tag="ln_t")
            nc.vector.tensor_tensor(t[:], x[:, m, :], neg_m[:], op=OP.add)
            nc.vector.tensor_tensor(t[:], t[:], inv_std[:], op=OP.mult)
            if res is None:
                nc.vector.tensor_scalar(out[:, m, :], t[:], g[:, m:m + 1],
                                        b[:, m:m + 1], op0=OP.mult, op1=OP.add)
            else:
                nc.vector.tensor_scalar(t[:], t[:], g[:, m:m + 1], b[:, m:m + 1],
                                        op0=OP.mult, op1=OP.add)
                nc.vector.tensor_tensor(out[:, m, :], t[:], res[:, m, :], op=OP.add)

        feat1 = None

        # =========================== CPE =================================
        with tc.tile_pool(name="hpool", bufs=1) as hpool:
            # zero the scatter accumulator
            with tc.tile_pool(name="zinit", bufs=1) as zp:
                zt = zp.tile([P, HROWS // P, C], BF16, tag="zt")
                nc.vector.memset(zt[:], 0.0)
                nc.sync.dma_start(
                    h_dram[:].rearrange("(a ki) e -> ki a e", ki=P), zt[:])

            fown = hpool.tile([P, CC, R], F32, tag="fown")
            nc.sync.dma_start(fown[:], featT_own[:])
            h1_sb = hpool.tile([P, CC, R], BF16, tag="h1")

            with tc.tile_pool(name="hbp", bufs=1) as hbp:
                h_base = hbp.tile([P, CC, R], BF16, tag="h_base")
                # ---- center tap: dense, identity-aligned, stays in SBUF ----
                with tc.tile_pool(name="w13p", bufs=1) as w13p, \
                     tc.tile_pool(name="cps", bufs=1, space="PSUM") as cps:
                    featbf = w13p.tile([P, CC, R], BF16, tag="featbf")
                    nc.vector.tensor_copy(featbf[:], fown[:])
                    w13 = wload(w13p, wcat[KCENTER], C, C, "w13")
                    for m in range(CC):
                        for g in range(CC):
                            ps = cps.tile([P, 512], F32, tag=f"cps{(m * CC + g) % 4}")
                            for kc in range(CC):
                                nc.tensor.matmul(
                                    ps[:], w13[:, kc, m * P:(m + 1) * P],
                                    featbf[:, kc, g * 512:(g + 1) * 512],
                                    start=(kc == 0), stop=(kc == CC - 1))
                            nc.vector.tensor_copy(
                                h_base[:, m, g * 512:(g + 1) * 512], ps[:])

                # ---- sparse taps: project k-group chunks, scatter-add ----
                ncpad = sum(nchunk.values()) * P
                si = hbp.tile([P, ncpad // 16], I16, tag="si")
                nc.sync.dma_start(si[:], sca_idx[:])
                # split the k-groups into ~4 u-load pieces (SBUF pressure)
                items = list(nchunk.items())
                total = sum(nk for _, nk in items)
                target = (total + 5) // 6
                groups, cur, acc = [], [], 0
                for k, nk in items:
                    cur.append((k, nk))
                    acc += nk
                    if acc >= target:
                        groups.append(cur)
                        cur, acc = [], 0
                if cur:
                    groups.append(cur)
                with (
                    tc.tile_pool(name="upool", bufs=2) as up,
                    tc.tile_pool(name="wstream", bufs=2) as wp,
                    tc.tile_pool(name="zbp", bufs=7) as zbp,
                    tc.tile_pool(name="cps2", bufs=2, space="PSUM") as cps2,
                ):
                    off = 0
                    for grp in groups:
                        gsize = sum(nk for _, nk in grp)
                        u_t = up.tile([P, CC, gsize * P], BF16, tag="u_t")
                        nc.sync.dma_start(
                            u_t[:], u_nc[:, :, off:off + gsize * P])
                        loff = 0
                        for k, nk in grp:
                            w_t = wload(wp, wcat[k], C, C, "w_t")
                            zb = zbp.tile([P, nk, C], BF16, tag="zb")
                            for j in range(nk):
                                ps = cps2.tile([P, C], F32, tag=f"ncps{j % 4}")
                                for kc in range(CC):
                                    nc.tensor.matmul(
                                        ps[:], u_t[:, kc, loff + j * P:
                                                   loff + (j + 1) * P],
                                        w_t[:, kc, :],
                                        start=(kc == 0), stop=(kc == CC - 1))
                                nc.vector.tensor_copy(zb[:, j, :], ps[:])
                            nc.gpsimd.dma_scatter_add(
                                h_dram[:], zb[:],
                                si[:, off // 16:(off + nk * P) // 16],
                                nk * P, nk * P, C, single_packet=False)
                            off += nk * P
                            loff += nk * P

                # ---- readback (transposed to FM) + combine + bias ----
                with tc.tile_pool(name="hrp", bufs=1) as hrp:
                    idt = hrp.tile([P, R // 16], I16, tag="idt")
                    nc.sync.dma_start(idt[:], ident_idx[:])
                    hr = hrp.tile([P, CC, R], BF16, tag="h_rest")
                    nc.gpsimd.dma_gather(hr[:], h_dram[:], idt[:], R, R, C,
                                         transpose=True, single_packet=False)
                    for m in range(CC):
                        nc.vector.scalar_tensor_tensor(
                            h1_sb[:, m, :], h_base[:, m, :],
                            pv["cpe_b"][:, m:m + 1],
                            hr[:, m, :], op0=OP.add, op1=OP.add)
                    tap("h1", h1_sb[:])

            # ---- cpe linear ----
            h2_sb = hpool.tile([P, CC, R], BF16, tag="h2")
            with (
                tc.tile_pool(name="linw", bufs=1) as lwp,
                tc.tile_pool(name="lin_ps", bufs=1, space="PSUM") as lps,
            ):
                lin_w_t = wload(lwp, lin_wT[:], C, C, "lin_w")
                for half in range(NHALF):
                    hp = [lps.tile([P, HALF], F32, tag=f"mm_ps{m}", name=f"lin_ps{m}") for m in range(CC)]
                    for kc in range(CC):
                        for m in range(CC):
                            for nn in range(N512):
                                sl = slice(nn * 512, (nn + 1) * 512)
                                hsl = slice(half * HALF + nn * 512,
                                            half * HALF + (nn + 1) * 512)
                                nc.tensor.matmul(
                                    hp[m][:, sl],
                                    lin_w_t[:, kc, m * P:(m + 1) * P],
                                    h1_sb[:, kc, hsl],
                                    start=(kc == 0), stop=(kc == CC - 1))
                    for m in range(CC):
                        nc.vector.tensor_scalar(
                            h2_sb[:, m, half * HALF:(half + 1) * HALF], hp[m][:],
                            pv["lin_b"][:, m:m + 1], None, op0=OP.add)

            feat1 = resid.tile([P, CC, R], F32, tag="resid")
            with tc.tile_pool(name="lnp1", bufs=1) as lnp:
                neg_m, inv_std = fm_ln_stats(lnp, h2_sb, x_is_f32=False)
                for m in range(CC):
                    fm_ln_apply(lnp, h2_sb, neg_m, inv_std, pv["cpe_ln_g"],
                                pv["cpe_ln_b"], feat1, m, res=fown)
            tap("feat1", feat1[:])

        # ===================== ln1 + qkv + attn + proj ===================
        with tc.tile_pool(name="attn", bufs=1) as ap_:
            q_sb = ap_.tile([P, CC, R], BF16, tag="q_sb")
            k_sb = ap_.tile([P, CC, R], BF16, tag="k_sb")
            v_sb = ap_.tile([P, R // P, NH, VD], F8, tag="v_sb")
            o_sb = ap_.tile([P, CC, R], F8, tag="o_sb")   # holds WS*o
            nc.vector.memset(v_sb[:], 1.0 / WS)           # denom col -> rec=WS/d

            with tc.tile_pool(name="x1p", bufs=1) as x1p:
                with tc.tile_pool(name="lnp2", bufs=1) as lnp:
                    neg_m, inv_std = fm_ln_stats(lnp, feat1, x_is_f32=True)
                    x1 = x1p.tile([P, CC, R], F8, tag="x1")
                    for m in range(CC):
                        fm_ln_apply(lnp, feat1, neg_m, inv_std, pv["ln1_g"],
                                    pv["ln1_b"], x1, m)
                    tap("x1", x1[:])

                with tc.tile_pool(name="qkv_ps", bufs=1, space="PSUM") as qps, \
                     tc.tile_pool(name="qkvw", bufs=2) as qwp:
                    for half in range(NHALF):
                        o = half * HALF
                        # q_b is host-scaled by SCALE already
                        for part, dst, bias, scl in [
                                (0, q_sb, pv["q_b"], SCALE / WS),
                                (1, k_sb, pv["k_b"], 1.0 / WS)]:
                            qkv_w_t = wload(qwp, qkv_wT[:, part * C:(part + 1) * C],
                                            C, C, "qkv_w_part", dt=F8)
                            pp_ = [qps.tile([P, HALF], F32, tag=f"mm_ps{m}", name=f"qkv_ps{m}")
                                   for m in range(CC)]
                            for kc in range(0, CC, 2):
                                for m in range(CC):
                                    for nn in range(N512):
                                        sl = slice(nn * 512, (nn + 1) * 512)
                                        nc.tensor.matmul(
                                            pp_[m][:, sl],
                                            qkv_w_t[:, kc:kc + 2,
                                                    m * P:(m + 1) * P],
                                            x1[:, kc:kc + 2, o + nn * 512:
                                               o + (nn + 1) * 512],
                                            start=(kc == 0), stop=(kc == CC - 2),
                                            perf_mode=DR)
                            for m in range(CC):
                                nc.vector.tensor_scalar(
                                    dst[:, m, o:o + HALF], pp_[m][:],
                                    bias[:, m:m + 1], scl,
                                    op0=OP.add, op1=OP.mult)
                        v_w_t = wload(qwp, qkv_wT[:, 2 * C:3 * C], C, C,
                                      "qkv_w_part", dt=F8)
                        for rc in range(HALF // P):
                            row0 = o + rc * P
                            vp = qps.tile([P, C], F32, tag=f"mm_ps{rc % 2}",
                                          name="vp")
                            for kc in range(0, CC, 2):
                                nc.tensor.matmul(
                                    vp[:], x1[:, kc:kc + 2, row0:row0 + P],
                                    v_w_t[:, kc:kc + 2, :],
                                    start=(kc == 0), stop=(kc == CC - 2),
                                    perf_mode=DR)
                            nc.vector.scalar_tensor_tensor(
                                v_sb[:, row0 // P, :, :HD],
                                vp[:].rearrange("p (h d) -> p h d", d=HD),
                                1.0 / WS,
                                v_b_t[:].rearrange("p (h d) -> p h d", d=HD),
                                op0=OP.mult, op1=OP.add)
                    tap("q", q_sb[:])
                    tap("k", k_sb[:])
                    tap("v", v_sb[:])

            with (
                tc.tile_pool(name="pT_pool", bufs=2) as ptp,
                tc.tile_pool(name="at_ps", bufs=1, space="PSUM") as aps,
                tc.tile_pool(name="at_ps2", bufs=2, space="PSUM") as aps2,
            ):
                for pt in range(NPATCH):
                    po = pt * KP
                    for h in range(NH):
                        hc, hpo = divmod(h * HD, P)
                        pT = ptp.tile([P, KP // P, KP], F8, tag="pT")
                        for jc in range(KP // P):
                            sps = aps2.tile([P, KP], F32, tag="s_ps")
                            for nn in range(KP // 512):
                                nc.tensor.matmul(
                                    sps[:, nn * 512:(nn + 1) * 512],
                                    k_sb[hpo:hpo + HD, hc,
                                         po + jc * P:po + (jc + 1) * P],
                                    q_sb[hpo:hpo + HD, hc,
                                         po + nn * 512:po + (nn + 1) * 512],
                                    start=True, stop=True)
                            nc.scalar.activation(pT[:, jc, :], sps[:], AF.Exp)
                        ops_ = aps.tile([P, KP], F32, tag="o_ps")
                        for jc in range(0, KP // P, 2):
                            for nn in range(KP // 512):
                                sl = slice(nn * 512, (nn + 1) * 512)
                                nc.tensor.matmul(
                                    ops_[:VD, sl],
                                    v_sb[:, (po + jc * P) // P:
                                         (po + jc * P) // P + 2, h, :],
                                    pT[:, jc:jc + 2, sl],
                                    start=(jc == 0), stop=(jc == KP // P - 2),
                                    perf_mode=DR)
                        # free the O psum bank early: copy to SBUF, then
                        # run the normalize tail off the PE critical path
                        ocp = ptp.tile([HD, KP], BF16, tag="ocp")
                        nc.vector.tensor_copy(ocp[:], ops_[:HD, :])
                        dcp = ptp.tile([1, KP], F32, tag="dcp")
                        nc.vector.tensor_copy(dcp[:], ops_[HD:HD + 1, :])
                        rec = ptp.tile([1, KP], F32, tag="rec")
                        nc.vector.reciprocal_approx_fast(rec[:], dcp[:])
                        rec_bf = ptp.tile([1, KP], BF16, tag="rec_bf")
                        nc.vector.tensor_copy(rec_bf[:], rec[:])
                        rps = aps.tile([HD, KP], F32, tag="rec_ps")
                        for nn in range(KP // 512):
                            sl = slice(nn * 512, (nn + 1) * 512)
                            nc.tensor.matmul(rps[:, sl], ones1_bf[:],
                                             rec_bf[:, sl], start=True, stop=True)
                        nc.vector.tensor_tensor(
                            o_sb[hpo:hpo + HD, hc, po:po + KP],
                            ocp[:], rps[:], op=OP.mult)
                tap("o", o_sb[:])

            feat2 = resid.tile([P, CC, R], F32, tag="resid")
            with (
                tc.tile_pool(name="projw", bufs=1) as pwp,
                tc.tile_pool(name="proj_tp", bufs=2) as ptp2,
                tc.tile_pool(name="proj_ps", bufs=1, space="PSUM") as pps,
            ):
                proj_w_t = wload(pwp, proj_wT[:], C, C, "proj_w", dt=F8)
                for half in range(NHALF):
                    o = half * HALF
                    pp_ = [pps.tile([P, HALF], F32, tag=f"mm_ps{m}", name=f"proj_ps{m}")
                           for m in range(CC)]
                    for kc in range(0, CC, 2):
                        for m in range(CC):
                            for nn in range(N512):
                                sl = slice(nn * 512, (nn + 1) * 512)
                                nc.tensor.matmul(
                                    pp_[m][:, sl],
                                    proj_w_t[:, kc:kc + 2, m * P:(m + 1) * P],
                                    o_sb[:, kc:kc + 2,
                                         o + nn * 512:o + (nn + 1) * 512],
                                    start=(kc == 0), stop=(kc == CC - 2),
                                    perf_mode=DR)
                    for m in range(CC):
                        tp_ = ptp2.tile([P, HALF], BF16, tag=f"tp{m % 2}")
                        nc.scalar.activation(
                            tp_[:], pp_[m][:], AF.Identity,
                            bias=pv["proj_b"][:, m:m + 1], scale=1.0 / (WS * WS))
                        nc.vector.tensor_tensor(
                            feat2[:, m, o:o + HALF], tp_[:],
                            feat1[:, m, o:o + HALF], op=OP.add)
            tap("feat2", feat2[:])

        # =============================== MLP =============================
        QH = 512  # row quarter
        with tc.tile_pool(name="mlp", bufs=1) as mp_:
            x2 = mp_.tile([P, CC, R], F8, tag="x2")
            with tc.tile_pool(name="lnp3", bufs=1) as lnp:
                neg_m, inv_std = fm_ln_stats(lnp, feat2, x_is_f32=True)
                for m in range(CC):
                    fm_ln_apply(lnp, feat2, neg_m, inv_std, pv["ln2_g"],
                                pv["ln2_b"], x2, m)

            gelu_f = AF.Gelu if gelu_exact else AF.Tanh
            fc1_w_t = wload(mp_, fc1_wT[:], C, 4 * C, "fc1_w", dt=F8)
            fc2_w_t = wload(mp_, fc2_wT[:], 4 * C, C, "fc2_w", dt=F8)
            with (
                tc.tile_pool(name="g_pool", bufs=2) as gp_,
                tc.tile_pool(name="out_pool", bufs=2) as op_,
                tc.tile_pool(name="mlp_tp", bufs=2) as mtp,
                tc.tile_pool(name="mlp_ps", bufs=2, space="PSUM") as mps,
            ):
                for quarter in range(R // QH):
                    o = quarter * QH
                    g_sb = gp_.tile([P, 4 * CC, QH], F8, tag="g_sb")
                    for mg in range(4):
                        fp = [mps.tile([P, QH], F32, tag=f"mm_ps{m}", name=f"mlp_ps{m}")
                              for m in range(CC)]
                        for kc in range(0, CC, 2):
                            for m in range(CC):
                                mm = mg * CC + m
                                nc.tensor.matmul(
                                    fp[m][:],
                                    fc1_w_t[:, kc:kc + 2, mm * P:(mm + 1) * P],
                                    x2[:, kc:kc + 2, o:o + QH],
                                    start=(kc == 0), stop=(kc == CC - 2),
                                    perf_mode=DR)
                        for m in range(CC):
                            mm = mg * CC + m
                            nc.scalar.activation(g_sb[:, mm, :], fp[m][:], gelu_f,
                                                 bias=pv["fc1_b"][:, mm:mm + 1],
                                                 scale=1.0 / WS)
                    f2 = [mps.tile([P, QH], F32, tag=f"mm_ps{m}", name=f"mlp_ps{m}") for m in range(CC)]
                    for kc in range(0, 4 * CC, 2):
                        for m in range(CC):
                            nc.tensor.matmul(
                                f2[m][:],
                                fc2_w_t[:, kc:kc + 2, m * P:(m + 1) * P],
                                g_sb[:, kc:kc + 2, :],
                                start=(kc == 0), stop=(kc == 4 * CC - 2),
                                perf_mode=DR)
                    out_q = op_.tile([P, CC, QH], F32, tag="out_q")
                    for m in range(CC):
                        tq_ = mtp.tile([P, QH], BF16, tag=f"tq{m % 2}")
                        nc.scalar.activation(
                            tq_[:], f2[m][:], AF.Identity,
                            bias=pv["fc2_b"][:, m:m + 1], scale=1.0 / WS)
                        nc.vector.tensor_tensor(
                            out_q[:, m, :], tq_[:],
                            feat2[:, m, o:o + QH], op=OP.add)
                    nc.sync.dma_start(outT[:, :, o:o + QH], out_q[:])

    nc.compile()
    return nc


# ====================== host-side preparation ======================

def prep_shared(inputs):
    f32 = np.float32
    bf = ml_dtypes.bfloat16
    f8 = mybir.dt.np(F8)
    ws = np.float32(WS)

    def pp(v):
        return np.ascontiguousarray(np.asarray(v, f32).reshape(-1, P).T)

    def w8(v):  # fp8 weight, pre-scaled so values sit in e4m3's normal range
        return np.ascontiguousarray(np.asarray(v, f32).T * ws).astype(f8)

    qkv_b = np.asarray(inputs["qkv_b"], f32)
    ident = np.arange(R, dtype=np.int16).reshape(-1, 16).T  # [16, R//16]
    return dict(
        ident_idx=np.ascontiguousarray(np.tile(ident, (P // 16, 1))),
        wcat=np.ascontiguousarray(
            np.transpose(np.asarray(inputs["cpe_w"], f32), (0, 2, 1))).astype(bf),
        lin_wT=np.ascontiguousarray(np.asarray(inputs["cpe_lin_w"], f32).T).astype(bf),
        qkv_wT=w8(inputs["qkv_w"]),
        proj_wT=w8(inputs["proj_w"]),
        fc1_wT=w8(inputs["fc1_w"]),
        fc2_wT=w8(inputs["fc2_w"]),
        cpe_b=pp(inputs["cpe_b"]), lin_b=pp(inputs["cpe_lin_b"]),
        cpe_ln_g=pp(inputs["cpe_ln_g"]), cpe_ln_b=pp(inputs["cpe_ln_b"]),
        ln1_g=pp(inputs["ln1_g"]), ln1_b=pp(inputs["ln1_b"]),
        ln2_g=pp(inputs["ln2_g"]), ln2_b=pp(inputs["ln2_b"]),
        q_b=pp(qkv_b[:C] * WS), k_b=pp(qkv_b[C:2 * C] * WS),
        v_b_rep=np.ascontiguousarray(np.broadcast_to(qkv_b[2 * C:], (P, C))),
        proj_b=pp(inputs["proj_b"]),
        fc1_b=pp(inputs["fc1_b"]), fc2_b=pp(inputs["fc2_b"]),
    )


def prep_core(inputs, core, nchunk):
    f32 = np.float32
    bf = ml_dtypes.bfloat16
    order = np.asarray(inputs["order"])
    feat = np.asarray(inputs["feat"], f32)
    nbr = np.asarray(inputs["neighbor_idx"])
    rows = order[core * R:(core + 1) * R]

    featT_own = np.ascontiguousarray(
        feat[rows].T.reshape(CC, P, R).transpose(1, 0, 2))

    nb = nbr[rows]
    srcs, dsts = [], []
    for k, nk in nchunk.items():
        v = np.nonzero(nb[:, k] >= 0)[0]
        src = np.full(nk * P, NFULL, np.int64)
        dst = np.full(nk * P, TRASH, np.int64)
        src[:len(v)] = nb[v, k]
        dst[:len(v)] = v
        srcs.append(src)
        dsts.append(dst)
    src_all = np.concatenate(srcs)
    dst_all = np.concatenate(dsts)

    featp = np.vstack([feat, np.zeros((1, C), f32)])
    u = featp[src_all]                                    # [NCPAD, C]
    u_fm = np.ascontiguousarray(
        u.T.reshape(CC, P, -1).transpose(1, 0, 2)).astype(bf)

    sca = dst_all.astype(np.int16).reshape(-1, 16).T      # [16, NCPAD//16]
    sca_idx = np.ascontiguousarray(np.tile(sca, (P // 16, 1)))
    return dict(featT_own=featT_own, u_nc=u_fm, sca_idx=sca_idx), rows


def unshard_out(res_outT):
    return np.ascontiguousarray(
        np.asarray(res_outT).transpose(1, 0, 2).reshape(C, R).T)


# ======================= public entry point =======================

_CACHED = {}


def get_program(inputs):
    """Build (or fetch) the program for these inputs' sparsity pattern."""
    nchunk = compute_nchunk(inputs["neighbor_idx"], inputs["order"])
    key = tuple(sorted(nchunk.items()))
    if key not in _CACHED:
        _CACHED[key] = build_program(nchunk)
    return _CACHED[key], nchunk


def kernel(**inputs) -> np.ndarray:
    """Full-input, full-output entry. Shards across 8 NeuronCores by
    serialized patches (2 per core), runs the Bass kernel, scatters the
    per-core outputs back to original point order."""
    from concourse.bass_utils import run_bass_kernel_spmd

    inputs = {k: np.asarray(v) for k, v in inputs.items()}
    nc, nchunk = get_program(inputs)
    sh = prep_shared(inputs)
    in_maps, rows_l = [], []
    for c in range(8):
        ci, rows = prep_core(inputs, c, nchunk)
        in_maps.append({**sh, **ci})
        rows_l.append(rows)

    res = None
    last_err = None
    for attempt in range(3):
        try:
            res = run_bass_kernel_spmd(nc, in_maps, core_ids=list(range(8))).results
            break
        except Exception as e:   # transient NRT/axon hiccups: retry
            last_err = e
            import time as _t
            _t.sleep(2.0)
    if res is None:
        raise last_err

    out = np.zeros((NFULL, C), np.float32)
    for c in range(8):
        out[rows_l[c]] = unshard_out(res[c]["outT"])
    return out


# revision 47
# speedup vs baseline: 53.9459x; 1.2180x over previous
"""Point-transformer block kernel for TRN2 (8-core data-parallel).

Core i handles serialized patches 2i,2i+1: rows = order[2048i:2048(i+1)].

CPE uses the ~17% sparsity of the 3x3x3 neighbor taps: the host ships,
per core, the *valid* (point, offset) pairs only — pre-gathered neighbor
features in feature-major layout, k-grouped and padded to 128-pair
chunks. Stage 1 projects each chunk with its offset's weights (PE);
stage 2 accumulates the projected rows into h via dma_scatter_add
(race-free: within one k-group every destination row is distinct).
The center tap (k=13, always valid, identity-aligned) is computed
densely into SBUF. h = center + gather_transpose(scattered part).

Activations feature-major (FM): X^T [128 (c%128), CC (c//128), rows];
matmuls lhsT=W^T-arranged weights. bf16 matmuls, f32 residual.
"""
from contextlib import ExitStack

import numpy as np
import ml_dtypes

import concourse.bacc as bacc
import concourse.bass as bass
import concourse.mybir as mybir
import concourse.tile as tile

P = 128
C = 512
CC = C // P
NH = 8
HD = 64
KP = 1024
R = 2048
NPATCH = R // KP
NKK = 27
KCENTER = 13
NFULL = 16384
EPS = 1e-5
SCALE = (C // NH) ** -0.5
F32 = mybir.dt.float32
BF16 = mybir.dt.bfloat16
F8 = mybir.dt.float8e4
I16 = mybir.dt.int16
AF = mybir.ActivationFunctionType
OP = mybir.AluOpType
DR = mybir.MatmulPerfMode.DoubleRow
WS = 16.0      # fp8 weight pre-scale (host multiplies weights by WS)
VD = 72        # v head width padded for DoubleRow (HD + denominator + pad)

HALF = 1024
NHALF = R // HALF
N512 = HALF // 512

HROWS = 17 * P          # h_dram rows: 2048 real + 128 trash/padding
TRASH = R               # scatter destination for padded pairs


def compute_nchunk(neighbor_idx, order):
    """Unified per-offset chunk counts (max over cores, 128-pair chunks)."""
    nbr = np.asarray(neighbor_idx)
    order = np.asarray(order)
    nchunk = {}
    for k in range(NKK):
        if k == KCENTER:
            continue
        mx = 0
        for c in range(8):
            rows = order[c * R:(c + 1) * R]
            mx = max(mx, int((nbr[rows, k] >= 0).sum()))
        if mx > 0:
            nchunk[k] = (mx + P - 1) // P
    return nchunk


def input_dram_specs(nchunk):
    """(name, shape, dtype) for every ExternalInput tensor."""
    ncpad = sum(nchunk.values()) * P
    specs = [
        ("featT_own", [P, CC, R], F32),
        ("u_nc", [P, CC, ncpad], BF16),
        ("sca_idx", [P, ncpad // 16], I16),
        ("ident_idx", [P, R // 16], I16),
        ("wcat", [NKK, C, C], BF16),
        ("lin_wT", [C, C], BF16),
        ("qkv_wT", [C, 3 * C], F8),
        ("proj_wT", [C, C], F8),
        ("fc1_wT", [C, 4 * C], F8),
        ("fc2_wT", [4 * C, C], F8),
    ]
    for nm, n in [("cpe_b", CC), ("lin_b", CC), ("cpe_ln_g", CC), ("cpe_ln_b", CC),
                  ("ln1_g", CC), ("ln1_b", CC), ("ln2_g", CC), ("ln2_b", CC),
                  ("q_b", CC), ("k_b", CC), ("proj_b", CC),
                  ("fc1_b", 4 * CC), ("fc2_b", CC)]:
        specs.append((nm, [P, n], F32))
    specs.append(("v_b_rep", [P, C], F32))
    return specs


def build_program(nchunk, gelu_exact=True, debug_taps=False):
    nc = bacc.Bacc("TRN2", target_bir_lowering=False, debug=False)

    dbg = {}

    def tap(name, ap):
        if not debug_taps:
            return
        t = nc.dram_tensor(f"dbg_{name}", list(ap.shape), ap.dtype,
                           kind="ExternalOutput")
        nc.sync.dma_start(t[:], ap)
        dbg[name] = t

    dram = {}
    for nm, shp, dt in input_dram_specs(nchunk):
        dram[nm] = nc.dram_tensor(nm, shp, dt, kind="ExternalInput")
    featT_own = dram["featT_own"]
    u_nc = dram["u_nc"]
    sca_idx = dram["sca_idx"]
    ident_idx = dram["ident_idx"]
    wcat = dram["wcat"]
    lin_wT, qkv_wT, proj_wT = dram["lin_wT"], dram["qkv_wT"], dram["proj_wT"]
    fc1_wT, fc2_wT = dram["fc1_wT"], dram["fc2_wT"]
    v_b_rep = dram["v_b_rep"]
    pvec_names = ["cpe_b", "lin_b", "cpe_ln_g", "cpe_ln_b", "ln1_g", "ln1_b",
                  "ln2_g", "ln2_b", "q_b", "k_b", "proj_b", "fc1_b", "fc2_b"]

    h_drams = [nc.dram_tensor(f"h_scratch{i}", [HROWS, C], BF16, kind="Internal")
               for i in range(2)]
    outT = nc.dram_tensor("outT", [P, CC, R], F32, kind="ExternalOutput")

    with tile.TileContext(nc) as tc, ExitStack() as ctx:
        pers = ctx.enter_context(tc.tile_pool(name="pers", bufs=1))
        resid = ctx.enter_context(tc.tile_pool(name="resid", bufs=2))

        pv = {}
        for nm in pvec_names:
            t = pers.tile(list(dram[nm].shape), F32, tag=f"pv_{nm}")
            nc.sync.dma_start(t[:], dram[nm][:])
            pv[nm] = t
        v_b_t = pers.tile([P, C], F32, tag="v_b")
        nc.sync.dma_start(v_b_t[:], v_b_rep[:])
        ones_bf = pers.tile([P, P], BF16, tag="ones_bf")
        nc.vector.memset(ones_bf[:], 1.0)
        ones_f = pers.tile([P, P], F32, tag="ones_f")
        nc.vector.memset(ones_f[:], 1.0)
        ones1_bf = pers.tile([1, HD], BF16, tag="ones1_bf")
        nc.vector.memset(ones1_bf[:], 1.0)
        eps_t = pers.tile([P, 1], F32, tag="eps_t")
        nc.vector.memset(eps_t[:], EPS)

        def wload(pool, dram_ap, kdim, ndim, tag, dt=BF16):
            t = pool.tile([P, kdim // P, ndim], dt, tag=tag)
            nc.sync.dma_start(t[:], dram_ap.rearrange("(ko ki) n -> ki ko n", ki=P))
            return t

        def fm_ln_stats(lnp, x, x_is_f32):
            with tc.tile_pool(name="ln_ps", bufs=1, space="PSUM") as lps:
                sums_ps = lps.tile([P, R], F32, tag="ln_sums")
                sqs_ps = lps.tile([P, R], F32, tag="ln_sqs")
                for half in range(NHALF):
                    o = half * HALF
                    sq = lnp.tile([P, CC, HALF], BF16, tag="ln_sq")
                    nc.scalar.activation(sq[:], x[:, :, o:o + HALF], AF.Square)
                    if x_is_f32:
                        # bf16 shadow so the sums matmul runs at 1 cyc/row
                        xb = lnp.tile([P, CC, HALF], BF16, tag="ln_xb")
                        nc.vector.tensor_copy(xb[:], x[:, :, o:o + HALF])
                    for kc in range(CC):
                        for nn in range(N512):
                            sl = slice(o + nn * 512, o + (nn + 1) * 512)
                            sli = slice(nn * 512, (nn + 1) * 512)
                            xsrc = xb[:, kc, sli] if x_is_f32 else x[:, kc, sl]
                            nc.tensor.matmul(sums_ps[:, sl], ones_bf[:], xsrc,
                                             start=(kc == 0), stop=(kc == CC - 1))
                            nc.tensor.matmul(sqs_ps[:, sl], ones_bf[:],
                                             sq[:, kc, sli],
                                             start=(kc == 0), stop=(kc == CC - 1))
                neg_m = lnp.tile([P, R], F32, tag="ln_negm")
                nc.vector.tensor_scalar(neg_m[:], sums_ps[:], -1.0 / C, None,
                                        op0=OP.mult)
                msq = lnp.tile([P, R], F32, tag="ln_tmp")
                nc.scalar.activation(msq[:], neg_m[:], AF.Square)
                var = lnp.tile([P, R], F32, tag="ln_tmp2")
                nc.vector.scalar_tensor_tensor(var[:], sqs_ps[:], 1.0 / C, msq[:],
                                               op0=OP.mult, op1=OP.subtract)
            std = lnp.tile([P, R], F32, tag="ln_tmp")
            nc.scalar.activation(std[:], var[:], AF.Sqrt, bias=eps_t[:])
            inv_std = lnp.tile([P, R], F32, tag="ln_istd")
            nc.vector.reciprocal_approx_fast(inv_std[:], std[:])
            return neg_m, inv_std

        def fm_ln_apply(lnp, x, neg_m, inv_std, g, b, out, m, res=None):
            t = lnp.tile([P, R], F32, tag="ln_t")
            nc.vector.tensor_tensor(t[:], x[:, m, :], neg_m[:], op=OP.add)
            nc.vector.tensor_tensor(t[:], t[:], inv_std[:], op=OP.mult)
            if res is None:
                nc.vector.tensor_scalar(out[:, m, :], t[:], g[:, m:m + 1],
                                        b[:, m:m + 1], op0=OP.mult, op1=OP.add)
            else:
                nc.vector.tensor_scalar(t[:], t[:], g[:, m:m + 1], b[:, m:m + 1],
                                        op0=OP.mult, op1=OP.add)
                nc.vector.tensor_tensor(out[:, m, :], t[:], res[:, m, :], op=OP.add)

        feat1 = None

        # =========================== CPE =================================
        with tc.tile_pool(name="hpool", bufs=1) as hpool:
            # zero the scatter accumulator
            with tc.tile_pool(name="zinit", bufs=1) as zp:
                zt = zp.tile([P, HROWS // P, C], BF16, tag="zt")
                nc.vector.memset(zt[:], 0.0)
                for hd in h_drams:
                    nc.sync.dma_start(
                        hd[:].rearrange("(a ki) e -> ki a e", ki=P), zt[:])

            fown = hpool.tile([P, CC, R], F32, tag="fown")
            nc.sync.dma_start(fown[:], featT_own[:])
            h1_sb = hpool.tile([P, CC, R], BF16, tag="h1")

            with tc.tile_pool(name="hbp", bufs=1) as hbp:
                h_base = hbp.tile([P, CC, R], BF16, tag="h_base")
                # ---- center tap: dense, identity-aligned, stays in SBUF ----
                with tc.tile_pool(name="w13p", bufs=1) as w13p, \
                     tc.tile_pool(name="cps", bufs=1, space="PSUM") as cps:
                    featbf = w13p.tile([P, CC, R], BF16, tag="featbf")
                    nc.vector.tensor_copy(featbf[:], fown[:])
                    w13 = wload(w13p, wcat[KCENTER], C, C, "w13")
                    for m in range(CC):
                        for g in range(CC):
                            ps = cps.tile([P, 512], F32, tag=f"cps{(m * CC + g) % 4}")
                            for kc in range(CC):
                                nc.tensor.matmul(
                                    ps[:], w13[:, kc, m * P:(m + 1) * P],
                                    featbf[:, kc, g * 512:(g + 1) * 512],
                                    start=(kc == 0), stop=(kc == CC - 1))
                            nc.vector.tensor_copy(
                                h_base[:, m, g * 512:(g + 1) * 512], ps[:])

                # ---- sparse taps: project k-group chunks, scatter-add ----
                ncpad = sum(nchunk.values()) * P
                si = hbp.tile([P, ncpad // 16], I16, tag="si")
                nc.sync.dma_start(si[:], sca_idx[:])
                # split the k-groups into ~4 u-load pieces (SBUF pressure)
                items = list(nchunk.items())
                total = sum(nk for _, nk in items)
                target = (total + 5) // 6
                groups, cur, acc = [], [], 0
                for k, nk in items:
                    cur.append((k, nk))
                    acc += nk
                    if acc >= target:
                        groups.append(cur)
                        cur, acc = [], 0
                if cur:
                    groups.append(cur)
                with (
                    tc.tile_pool(name="upool", bufs=2) as up,
                    tc.tile_pool(name="wstream", bufs=2) as wp,
                    tc.tile_pool(name="zbp", bufs=7) as zbp,
                    tc.tile_pool(name="cps2", bufs=2, space="PSUM") as cps2,
                ):
                    off = 0
                    gi = 0
                    for grp in groups:
                        gsize = sum(nk for _, nk in grp)
                        u_t = up.tile([P, CC, gsize * P], BF16, tag="u_t")
                        nc.sync.dma_start(
                            u_t[:], u_nc[:, :, off:off + gsize * P])
                        loff = 0
                        for k, nk in grp:
                            w_t = wload(wp, wcat[k], C, C, "w_t")
                            zb = zbp.tile([P, nk, C], BF16, tag="zb")
                            for j in range(nk):
                                ps = cps2.tile([P, C], F32, tag=f"ncps{j % 4}")
                                for kc in range(CC):
                                    nc.tensor.matmul(
                                        ps[:], u_t[:, kc, loff + j * P:
                                                   loff + (j + 1) * P],
                                        w_t[:, kc, :],
                                        start=(kc == 0), stop=(kc == CC - 1))
                                nc.vector.tensor_copy(zb[:, j, :], ps[:])
                            nc.gpsimd.dma_scatter_add(
                                h_drams[gi % 2][:], zb[:],
                                si[:, off // 16:(off + nk * P) // 16],
                                nk * P, nk * P, C, single_packet=False)
                            off += nk * P
                            loff += nk * P
                            gi += 1

                # ---- readback (transposed to FM) + combine + bias ----
                with tc.tile_pool(name="hrp", bufs=1) as hrp:
                    idt = hrp.tile([P, R // 16], I16, tag="idt")
                    nc.sync.dma_start(idt[:], ident_idx[:])
                    hrs = []
                    for i, hd in enumerate(h_drams):
                        hr = hrp.tile([P, CC, R], BF16, tag=f"h_rest{i}")
                        nc.gpsimd.dma_gather(hr[:], hd[:], idt[:], R, R, C,
                                             transpose=True,
                                             single_packet=False)
                        hrs.append(hr)
                    for m in range(CC):
                        nc.vector.scalar_tensor_tensor(
                            h1_sb[:, m, :], h_base[:, m, :],
                            pv["cpe_b"][:, m:m + 1],
                            hrs[0][:, m, :], op0=OP.add, op1=OP.add)
                        nc.vector.tensor_tensor(
                            h1_sb[:, m, :], h1_sb[:, m, :], hrs[1][:, m, :],
                            op=OP.add)
                    tap("h1", h1_sb[:])

            # ---- cpe linear ----
            h2_sb = hpool.tile([P, CC, R], BF16, tag="h2")
            with (
                tc.tile_pool(name="linw", bufs=1) as lwp,
                tc.tile_pool(name="lin_ps", bufs=1, space="PSUM") as lps,
            ):
                lin_w_t = wload(lwp, lin_wT[:], C, C, "lin_w")
                for half in range(NHALF):
                    hp = [lps.tile([P, HALF], F32, tag=f"mm_ps{m}", name=f"lin_ps{m}") for m in range(CC)]
                    for kc in range(CC):
                        for m in range(CC):
                            for nn in range(N512):
                                sl = slice(nn * 512, (nn + 1) * 512)
                                hsl = slice(half * HALF + nn * 512,
                                            half * HALF + (nn + 1) * 512)
                                nc.tensor.matmul(
                                    hp[m][:, sl],
                                    lin_w_t[:, kc, m * P:(m + 1) * P],
                                    h1_sb[:, kc, hsl],
                                    start=(kc == 0), stop=(kc == CC - 1))
                    for m in range(CC):
                        nc.vector.tensor_scalar(
                            h2_sb[:, m, half * HALF:(half + 1) * HALF], hp[m][:],
                            pv["lin_b"][:, m:m + 1], None, op0=OP.add)

            feat1 = resid.tile([P, CC, R], F32, tag="resid")
            with tc.tile_pool(name="lnp1", bufs=1) as lnp:
                neg_m, inv_std = fm_ln_stats(lnp, h2_sb, x_is_f32=False)
                for m in range(CC):
                    fm_ln_apply(lnp, h2_sb, neg_m, inv_std, pv["cpe_ln_g"],
                                pv["cpe_ln_b"], feat1, m, res=fown)
            tap("feat1", feat1[:])

        # ===================== ln1 + qkv + attn + proj ===================
        with tc.tile_pool(name="attn", bufs=1) as ap_:
            q_sb = ap_.tile([P, CC, R], BF16, tag="q_sb")
            k_sb = ap_.tile([P, CC, R], BF16, tag="k_sb")
            v_sb = ap_.tile([P, R // P, NH, VD], F8, tag="v_sb")
            o_sb = ap_.tile([P, CC, R], F8, tag="o_sb")   # holds WS*o
            nc.vector.memset(v_sb[:], 1.0 / WS)           # denom col -> rec=WS/d

            with tc.tile_pool(name="x1p", bufs=1) as x1p:
                with tc.tile_pool(name="lnp2", bufs=1) as lnp:
                    neg_m, inv_std = fm_ln_stats(lnp, feat1, x_is_f32=True)
                    x1 = x1p.tile([P, CC, R], F8, tag="x1")
                    for m in range(CC):
                        fm_ln_apply(lnp, feat1, neg_m, inv_std, pv["ln1_g"],
                                    pv["ln1_b"], x1, m)
                    tap("x1", x1[:])

                with tc.tile_pool(name="qkv_ps", bufs=1, space="PSUM") as qps, \
                     tc.tile_pool(name="qkvw", bufs=2) as qwp:
                    for half in range(NHALF):
                        o = half * HALF
                        # q_b is host-scaled by SCALE already
                        for part, dst, bias, scl in [
                                (0, q_sb, pv["q_b"], SCALE / WS),
                                (1, k_sb, pv["k_b"], 1.0 / WS)]:
                            qkv_w_t = wload(qwp, qkv_wT[:, part * C:(part + 1) * C],
                                            C, C, "qkv_w_part", dt=F8)
                            pp_ = [qps.tile([P, HALF], F32, tag=f"mm_ps{m}", name=f"qkv_ps{m}")
                                   for m in range(CC)]
                            for kc in range(0, CC, 2):
                                for m in range(CC):
                                    for nn in range(N512):
                                        sl = slice(nn * 512, (nn + 1) * 512)
                                        nc.tensor.matmul(
                                            pp_[m][:, sl],
                                            qkv_w_t[:, kc:kc + 2,
                                                    m * P:(m + 1) * P],
                                            x1[:, kc:kc + 2, o + nn * 512:
                                               o + (nn + 1) * 512],
                                            start=(kc == 0), stop=(kc == CC - 2),
                                            perf_mode=DR)
                            for m in range(CC):
                                nc.vector.tensor_scalar(
                                    dst[:, m, o:o + HALF], pp_[m][:],
                                    bias[:, m:m + 1], scl,
                                    op0=OP.add, op1=OP.mult)
                        v_w_t = wload(qwp, qkv_wT[:, 2 * C:3 * C], C, C,
                                      "qkv_w_part", dt=F8)
                        for rc in range(HALF // P):
                            row0 = o + rc * P
                            vp = qps.tile([P, C], F32, tag=f"mm_ps{rc % 2}",
                                          name="vp")
                            for kc in range(0, CC, 2):
                                nc.tensor.matmul(
                                    vp[:], x1[:, kc:kc + 2, row0:row0 + P],
                                    v_w_t[:, kc:kc + 2, :],
                                    start=(kc == 0), stop=(kc == CC - 2),
                                    perf_mode=DR)
                            nc.vector.scalar_tensor_tensor(
                                v_sb[:, row0 // P, :, :HD],
                                vp[:].rearrange("p (h d) -> p h d", d=HD),
                                1.0 / WS,
                                v_b_t[:].rearrange("p (h d) -> p h d", d=HD),
                                op0=OP.mult, op1=OP.add)
                    tap("q", q_sb[:])
                    tap("k", k_sb[:])
                    tap("v", v_sb[:])

            with (
                tc.tile_pool(name="pT_pool", bufs=2) as ptp,
                tc.tile_pool(name="at_ps", bufs=1, space="PSUM") as aps,
                tc.tile_pool(name="at_ps2", bufs=2, space="PSUM") as aps2,
            ):
                def finish_head(st):
                    # normalize tail of a previous head: deferred past the
                    # next head's QK so the PE never waits on the DVE chain
                    ocp, rec_bf, hc, hpo, po = st
                    rps = aps.tile([HD, KP], F32, tag="rec_ps")
                    for nn in range(KP // 512):
                        sl = slice(nn * 512, (nn + 1) * 512)
                        nc.tensor.matmul(rps[:, sl], ones1_bf[:],
                                         rec_bf[:, sl], start=True, stop=True)
                    nc.vector.tensor_tensor(
                        o_sb[hpo:hpo + HD, hc, po:po + KP],
                        ocp[:], rps[:], op=OP.mult)

                pending = None
                for pt in range(NPATCH):
                    po = pt * KP
                    for h in range(NH):
                        hc, hpo = divmod(h * HD, P)
                        pT = ptp.tile([P, KP // P, KP], F8, tag="pT")
                        for jc in range(KP // P):
                            sps = aps2.tile([P, KP], F32, tag="s_ps")
                            for nn in range(KP // 512):
                                nc.tensor.matmul(
                                    sps[:, nn * 512:(nn + 1) * 512],
                                    k_sb[hpo:hpo + HD, hc,
                                         po + jc * P:po + (jc + 1) * P],
                                    q_sb[hpo:hpo + HD, hc,
                                         po + nn * 512:po + (nn + 1) * 512],
                                    start=True, stop=True)
                            nc.scalar.activation(pT[:, jc, :], sps[:], AF.Exp)
                        if pending is not None:
                            finish_head(pending)
                        ops_ = aps.tile([P, KP], F32, tag="o_ps")
                        for jc in range(0, KP // P, 2):
                            for nn in range(KP // 512):
                                sl = slice(nn * 512, (nn + 1) * 512)
                                nc.tensor.matmul(
                                    ops_[:VD, sl],
                                    v_sb[:, (po + jc * P) // P:
                                         (po + jc * P) // P + 2, h, :],
                                    pT[:, jc:jc + 2, sl],
                                    start=(jc == 0), stop=(jc == KP // P - 2),
                                    perf_mode=DR)
                        ocp = ptp.tile([HD, KP], BF16, tag="ocp")
                        nc.vector.tensor_copy(ocp[:], ops_[:HD, :])
                        dcp = ptp.tile([1, KP], F32, tag="dcp")
                        nc.vector.tensor_copy(dcp[:], ops_[HD:HD + 1, :])
                        rec = ptp.tile([1, KP], F32, tag="rec")
                        nc.vector.reciprocal_approx_fast(rec[:], dcp[:])
                        rec_bf = ptp.tile([1, KP], BF16, tag="rec_bf")
                        nc.vector.tensor_copy(rec_bf[:], rec[:])
                        pending = (ocp, rec_bf, hc, hpo, po)
                finish_head(pending)
                tap("o", o_sb[:])

            feat2 = resid.tile([P, CC, R], F32, tag="resid")
            with (
                tc.tile_pool(name="projw", bufs=1) as pwp,
                tc.tile_pool(name="proj_tp", bufs=2) as ptp2,
                tc.tile_pool(name="proj_ps", bufs=1, space="PSUM") as pps,
            ):
                proj_w_t = wload(pwp, proj_wT[:], C, C, "proj_w", dt=F8)
                for half in range(NHALF):
                    o = half * HALF
                    pp_ = [pps.tile([P, HALF], F32, tag=f"mm_ps{m}", name=f"proj_ps{m}")
                           for m in range(CC)]
                    for kc in range(0, CC, 2):
                        for m in range(CC):
                            for nn in range(N512):
                                sl = slice(nn * 512, (nn + 1) * 512)
                                nc.tensor.matmul(
                                    pp_[m][:, sl],
                                    proj_w_t[:, kc:kc + 2, m * P:(m + 1) * P],
                                    o_sb[:, kc:kc + 2,
                                         o + nn * 512:o + (nn + 1) * 512],
                                    start=(kc == 0), stop=(kc == CC - 2),
                                    perf_mode=DR)
                    for m in range(CC):
                        tp_ = ptp2.tile([P, HALF], BF16, tag=f"tp{m % 2}")
                        nc.scalar.activation(
                            tp_[:], pp_[m][:], AF.Identity,
                            bias=pv["proj_b"][:, m:m + 1], scale=1.0 / (WS * WS))
                        nc.vector.tensor_tensor(
                            feat2[:, m, o:o + HALF], tp_[:],
                            feat1[:, m, o:o + HALF], op=OP.add)
            tap("feat2", feat2[:])

        # =============================== MLP =============================
        QH = 512  # row quarter
        with tc.tile_pool(name="mlp", bufs=1) as mp_:
            x2 = mp_.tile([P, CC, R], F8, tag="x2")
            with tc.tile_pool(name="lnp3", bufs=1) as lnp:
                neg_m, inv_std = fm_ln_stats(lnp, feat2, x_is_f32=True)
                for m in range(CC):
                    fm_ln_apply(lnp, feat2, neg_m, inv_std, pv["ln2_g"],
                                pv["ln2_b"], x2, m)

            gelu_f = AF.Gelu if gelu_exact else AF.Tanh
            fc1_w_t = wload(mp_, fc1_wT[:], C, 4 * C, "fc1_w", dt=F8)
            fc2_w_t = wload(mp_, fc2_wT[:], 4 * C, C, "fc2_w", dt=F8)
            with (
                tc.tile_pool(name="g_pool", bufs=2) as gp_,
                tc.tile_pool(name="out_pool", bufs=2) as op_,
                tc.tile_pool(name="mlp_tp", bufs=2) as mtp,
                tc.tile_pool(name="mlp_ps", bufs=2, space="PSUM") as mps,
            ):
                for quarter in range(R // QH):
                    o = quarter * QH
                    g_sb = gp_.tile([P, 4 * CC, QH], F8, tag="g_sb")
                    for mg in range(4):
                        fp = [mps.tile([P, QH], F32, tag=f"mm_ps{m}", name=f"mlp_ps{m}")
                              for m in range(CC)]
                        for kc in range(0, CC, 2):
                            for m in range(CC):
                                mm = mg * CC + m
                                nc.tensor.matmul(
                                    fp[m][:],
                                    fc1_w_t[:, kc:kc + 2, mm * P:(mm + 1) * P],
                                    x2[:, kc:kc + 2, o:o + QH],
                                    start=(kc == 0), stop=(kc == CC - 2),
                                    perf_mode=DR)
                        for m in range(CC):
                            mm = mg * CC + m
                            nc.scalar.activation(g_sb[:, mm, :], fp[m][:], gelu_f,
                                                 bias=pv["fc1_b"][:, mm:mm + 1],
                                                 scale=1.0 / WS)
                    f2 = [mps.tile([P, QH], F32, tag=f"mm_ps{m}", name=f"mlp_ps{m}") for m in range(CC)]
                    for kc in range(0, 4 * CC, 2):
                        for m in range(CC):
                            nc.tensor.matmul(
                                f2[m][:],
                                fc2_w_t[:, kc:kc + 2, m * P:(m + 1) * P],
                                g_sb[:, kc:kc + 2, :],
                                start=(kc == 0), stop=(kc == 4 * CC - 2),
                                perf_mode=DR)
                    out_q = op_.tile([P, CC, QH], F32, tag="out_q")
                    for m in range(CC):
                        tq_ = mtp.tile([P, QH], BF16, tag=f"tq{m % 2}")
                        nc.scalar.activation(
                            tq_[:], f2[m][:], AF.Identity,
                            bias=pv["fc2_b"][:, m:m + 1], scale=1.0 / WS)
                        nc.vector.tensor_tensor(
                            out_q[:, m, :], tq_[:],
                            feat2[:, m, o:o + QH], op=OP.add)
                    nc.sync.dma_start(outT[:, :, o:o + QH], out_q[:])

    nc.compile()
    return nc


# ====================== host-side preparation ======================

def prep_shared(inputs):
    f32 = np.float32
    bf = ml_dtypes.bfloat16
    f8 = mybir.dt.np(F8)
    ws = np.float32(WS)

    def pp(v):
        return np.ascontiguousarray(np.asarray(v, f32).reshape(-1, P).T)

    def w8(v):  # fp8 weight, pre-scaled so values sit in e4m3's normal range
        return np.ascontiguousarray(np.asarray(v, f32).T * ws).astype(f8)

    qkv_b = np.asarray(inputs["qkv_b"], f32)
    ident = np.arange(R, dtype=np.int16).reshape(-1, 16).T  # [16, R//16]
    return dict(
        ident_idx=np.ascontiguousarray(np.tile(ident, (P // 16, 1))),
        wcat=np.ascontiguousarray(
            np.transpose(np.asarray(inputs["cpe_w"], f32), (0, 2, 1))).astype(bf),
        lin_wT=np.ascontiguousarray(np.asarray(inputs["cpe_lin_w"], f32).T).astype(bf),
        qkv_wT=w8(inputs["qkv_w"]),
        proj_wT=w8(inputs["proj_w"]),
        fc1_wT=w8(inputs["fc1_w"]),
        fc2_wT=w8(inputs["fc2_w"]),
        cpe_b=pp(inputs["cpe_b"]), lin_b=pp(inputs["cpe_lin_b"]),
        cpe_ln_g=pp(inputs["cpe_ln_g"]), cpe_ln_b=pp(inputs["cpe_ln_b"]),
        ln1_g=pp(inputs["ln1_g"]), ln1_b=pp(inputs["ln1_b"]),
        ln2_g=pp(inputs["ln2_g"]), ln2_b=pp(inputs["ln2_b"]),
        q_b=pp(qkv_b[:C] * WS), k_b=pp(qkv_b[C:2 * C] * WS),
        v_b_rep=np.ascontiguousarray(np.broadcast_to(qkv_b[2 * C:], (P, C))),
        proj_b=pp(inputs["proj_b"]),
        fc1_b=pp(inputs["fc1_b"]), fc2_b=pp(inputs["fc2_b"]),
    )


def prep_core(inputs, core, nchunk):
    f32 = np.float32
    bf = ml_dtypes.bfloat16
    order = np.asarray(inputs["order"])
    feat = np.asarray(inputs["feat"], f32)
    nbr = np.asarray(inputs["neighbor_idx"])
    rows = order[core * R:(core + 1) * R]

    featT_own = np.ascontiguousarray(
        feat[rows].T.reshape(CC, P, R).transpose(1, 0, 2))

    nb = nbr[rows]
    srcs, dsts = [], []
    for k, nk in nchunk.items():
        v = np.nonzero(nb[:, k] >= 0)[0]
        src = np.full(nk * P, NFULL, np.int64)
        dst = np.full(nk * P, TRASH, np.int64)
        src[:len(v)] = nb[v, k]
        dst[:len(v)] = v
        srcs.append(src)
        dsts.append(dst)
    src_all = np.concatenate(srcs)
    dst_all = np.concatenate(dsts)

    featp = np.vstack([feat, np.zeros((1, C), f32)])
    u = featp[src_all]                                    # [NCPAD, C]
    u_fm = np.ascontiguousarray(
        u.T.reshape(CC, P, -1).transpose(1, 0, 2)).astype(bf)

    sca = dst_all.astype(np.int16).reshape(-1, 16).T      # [16, NCPAD//16]
    sca_idx = np.ascontiguousarray(np.tile(sca, (P // 16, 1)))
    return dict(featT_own=featT_own, u_nc=u_fm, sca_idx=sca_idx), rows


def unshard_out(res_outT):
    return np.ascontiguousarray(
        np.asarray(res_outT).transpose(1, 0, 2).reshape(C, R).T)


# ======================= public entry point =======================

_CACHED = {}


def get_program(inputs):
    """Build (or fetch) the program for these inputs' sparsity pattern."""
    nchunk = compute_nchunk(inputs["neighbor_idx"], inputs["order"])
    key = tuple(sorted(nchunk.items()))
    if key not in _CACHED:
        _CACHED[key] = build_program(nchunk)
    return _CACHED[key], nchunk


def kernel(**inputs) -> np.ndarray:
    """Full-input, full-output entry. Shards across 8 NeuronCores by
    serialized patches (2 per core), runs the Bass kernel, scatters the
    per-core outputs back to original point order."""
    from concourse.bass_utils import run_bass_kernel_spmd

    inputs = {k: np.asarray(v) for k, v in inputs.items()}
    nc, nchunk = get_program(inputs)
    sh = prep_shared(inputs)
    in_maps, rows_l = [], []
    for c in range(8):
        ci, rows = prep_core(inputs, c, nchunk)
        in_maps.append({**sh, **ci})
        rows_l.append(rows)

    res = None
    last_err = None
    for attempt in range(3):
        try:
            res = run_bass_kernel_spmd(nc, in_maps, core_ids=list(range(8))).results
            break
        except Exception as e:   # transient NRT/axon hiccups: retry
            last_err = e
            import time as _t
            _t.sleep(2.0)
    if res is None:
        raise last_err

    out = np.zeros((NFULL, C), np.float32)
    for c in range(8):
        out[rows_l[c]] = unshard_out(res[c]["outT"])
    return out
